# revision 7
# baseline (speedup 1.0000x reference)
"""Trainium2 Bass kernel for nn_MemoryLayer.

Structure:
  - GRU-style scan (T steps) with two merged recurrent branches (h: argmax->
    gather feedback; m: memory-driven), replicated on all 8 cores.
  - Phase 2 (projection through lin_w + cosine similarity over heads) is
    O-sharded: each core gets a 1024-row slice of lin_w and produces a
    (T*32, 128) slice of the output.

All matmuls put the contraction dim on partitions with the small "batch"
operand stationary (lhsT) and the weight matrix moving (rhs).
"""

import numpy as np
import ml_dtypes

import concourse.bass as bass
import concourse.mybir as mybir
import concourse.tile as tile
import bass_rust
from concourse.bass import ts, ds, IndirectOffsetOnAxis
from concourse.bass_utils import run_bass_kernel_spmd
from concourse.masks import make_identity
from concourse.tile_rust import add_dep_helper

F32 = mybir.dt.float32
BF16 = mybir.dt.bfloat16
U32 = mybir.dt.uint32
AF = mybir.ActivationFunctionType
ALU = mybir.AluOpType

EPS_BN = 1e-5
EPS_COS = 1e-8
SLOPE = 0.2

B = 32
H = 512
O = 1024
HEADS = 8
S = 2 * H          # 1024
G = 3 * H          # 1536
NCORES = 8
OSH = O // NCORES  # o's per core (128)


def _lrelu_np(x):
    return np.where(x >= 0, x, SLOPE * x)


def _split_multiwait_ctrl(nc):
    """This walrus build allows only one sync-wait per instruction; hoist all
    but one wait of any multi-wait instruction onto preceding single-wait
    NoOps on the same engine (same per-engine queue => same semantics)."""
    for f in nc.m.functions:
        for bb in f.blocks:
            new_list = []
            changed = False
            for ins in bb.instructions:
                si = ins.sync_info
                nw = len(si.on_wait) if si and si.on_wait else 0
                if nw > 1:
                    waits = list(si.on_wait)
                    for j, w in enumerate(waits[:-1]):
                        d2 = mybir.InstNoOp(
                            name=f"{ins.name}-sw{j}", ins=[], outs=[]
                        )
                        d2.engine = ins.engine
                        d2.sync_info = bass_rust.SyncInfo(
                            on_wait=[w], on_update=[]
                        )
                        new_list.append(d2)
                    si.on_wait = [waits[-1]]
                    changed = True
                new_list.append(ins)
            if changed:
                bb.instructions = new_list


def _bcast_ap(handle, nrows, ncols):
    ap0 = handle[:]
    return bass.AP(tensor=ap0.tensor, offset=ap0.offset,
                   ap=[[0, nrows], [1, ncols]])


def build_program(T, flags):
    have_bhh_n, have_h2ob, have_linb, have_bn2b, have_bn3b = flags
    R = B * T  # rows per branch
    nc = bass.Bass()

    # ---------------- DRAM I/O ----------------
    d_wl = nc.dram_tensor("wl", (128, 4, G), F32, kind="ExternalInput")
    d_whh = nc.dram_tensor("whh", (128, 4, G), F32, kind="ExternalInput")
    d_ho = nc.dram_tensor("ho", (128, 4, O), F32, kind="ExternalInput")
    d_z2g = nc.dram_tensor("z2g", (B, G), F32, kind="ExternalInput")
    d_gim = nc.dram_tensor("gim", (T, G), F32, kind="ExternalInput")
    d_e2 = nc.dram_tensor("e2", (O, H), F32, kind="ExternalInput")
    d_st0 = nc.dram_tensor("st0", (2 * B, H), F32, kind="ExternalInput")
    d_st0t = nc.dram_tensor("st0t", (128, 4, 2 * B), F32, kind="ExternalInput")
    d_bn2g = nc.dram_tensor("bn2g", (128, 4), F32, kind="ExternalInput")
    d_bn2b = nc.dram_tensor("bn2b", (128, 4), F32, kind="ExternalInput")
    d_bn3g8 = nc.dram_tensor("bn3g8", (128, 8), F32, kind="ExternalInput")
    d_bn3b8 = nc.dram_tensor("bn3b8", (128, 8), F32, kind="ExternalInput")
    d_mask = nc.dram_tensor("maskm", (1, 2 * B), F32, kind="ExternalInput")
    d_i32p = nc.dram_tensor("i32p", (B, 2 * B), F32, kind="ExternalInput")
    d_lw = nc.dram_tensor("linw", (128, 8, O), BF16, kind="ExternalInput")
    d_bhh_n = (
        nc.dram_tensor("bhhn", (1, H), F32, kind="ExternalInput")
        if have_bhh_n else None
    )
    d_h2ob = (
        nc.dram_tensor("h2ob", (1, O), F32, kind="ExternalInput")
        if have_h2ob else None
    )
    d_linb = (
        nc.dram_tensor("linb", (1, O), F32, kind="ExternalInput")
        if have_linb else None
    )

    d_ph = nc.dram_tensor("ph", (R, O), BF16)
    d_pm = nc.dram_tensor("pm", (R, O), BF16)
    d_out = nc.dram_tensor("out", (R, OSH), F32, kind="ExternalOutput")

    ph_stores = [None] * T
    pm_stores = [None] * T

    with tile.TileContext(nc) as tc:
        with tc.tile_pool(name="consts", bufs=1) as cp:
            # ---------- persistent constants ----------
            cWl = cp.tile([128, 4, G], F32)
            nc.sync.dma_start(cWl[:], d_wl[:])
            cWhh = cp.tile([128, 4, G], F32)
            nc.sync.dma_start(cWhh[:], d_whh[:])
            cHo = cp.tile([128, 4, O], F32)
            nc.sync.dma_start(cHo[:], d_ho[:])
            cZ2G = cp.tile([B, G], F32)
            nc.sync.dma_start(cZ2G[:], d_z2g[:])
            cSt = cp.tile([2 * B, H], F32)
            nc.sync.dma_start(cSt[:], d_st0[:])
            cStT = cp.tile([128, 4, 2 * B], F32)
            nc.sync.dma_start(cStT[:], d_st0t[:])
            cBn2g = cp.tile([128, 4], F32)
            nc.sync.dma_start(cBn2g[:], d_bn2g[:])
            cBn2b = cp.tile([128, 4], F32)
            nc.sync.dma_start(cBn2b[:], d_bn2b[:])
            cBn3g8 = cp.tile([128, 8], F32)
            nc.sync.dma_start(cBn3g8[:], d_bn3g8[:])
            cBn3b8 = cp.tile([128, 8], F32)
            nc.sync.dma_start(cBn3b8[:], d_bn3b8[:])
            cMask = cp.tile([1, 2 * B], F32)
            nc.sync.dma_start(cMask[:], d_mask[:])
            cI32p = cp.tile([B, 2 * B], F32)
            nc.sync.dma_start(cI32p[:], d_i32p[:])
            cLw = cp.tile([128, 8, O], BF16)
            nc.sync.dma_start(cLw[:], d_lw[:])
            cIdent = cp.tile([128, 128], F32)
            make_identity(nc, cIdent[:])
            cEps = cp.tile([128, 1], F32)
            nc.vector.memset(cEps[:], EPS_BN)
            cPrev = cp.tile([B, 1], U32)
            nc.vector.memset(cPrev[:], O - 1)
            cBhhN = None
            if d_bhh_n is not None:
                cBhhN = cp.tile([2 * B, H], F32)
                nc.sync.dma_start(cBhhN[:], _bcast_ap(d_bhh_n, 2 * B, H))
            cH2ob = None
            if d_h2ob is not None:
                cH2ob = cp.tile([2 * B, O], F32)
                nc.sync.dma_start(cH2ob[:], _bcast_ap(d_h2ob, 2 * B, O))
            cLinb = None
            if d_linb is not None:
                cLinb = cp.tile([128, O], F32)
                nc.sync.dma_start(cLinb[:], _bcast_ap(d_linb, 128, O))

            prev_ap = cPrev[:, 0:1]

            # =================== SCAN ===================
            with (
                tc.tile_pool(name="sb", bufs=2) as sb,
                tc.tile_pool(name="sbl", bufs=2) as sbl,
                tc.tile_pool(name="ps_gh", bufs=1, space="PSUM") as ps_gh,
                tc.tile_pool(name="ps_gin", bufs=1, space="PSUM") as ps_gin,
                tc.tile_pool(name="ps_log", bufs=1, space="PSUM") as ps_log,
                tc.tile_pool(name="ps_tr", bufs=1, space="PSUM") as ps_tr,
            ):
                for t in range(T):
                    # gim row for this step (prefetchable, no deps)
                    gmt = sb.tile([1, G], F32, tag="gmt")
                    nc.sync.dma_start(gmt[:], d_gim[t:t + 1, :])
                    # ---- B: gh = [h;m] @ w_hh.T (+ rz const injections)
                    gh = ps_gh.tile([2 * B, G], F32, tag="gh")
                    for k in range(4):
                        for n in range(3):
                            nc.tensor.matmul(
                                gh[:, ts(n, 512)], lhsT=cStT[:, k, :],
                                rhs=cWhh[:, k, ts(n, 512)],
                                start=(k == 0),
                                stop=(k == 3 and n == 2),
                            )
                    for n in range(2):
                        nc.tensor.matmul(
                            gh[:, ts(n, 512)], lhsT=cI32p[:],
                            rhs=cZ2G[:, ts(n, 512)], start=False, stop=False,
                        )
                    # ---- gin: n-gate input-side pre-activation
                    gin = ps_gin.tile([2 * B, H], F32, tag="gin")
                    nc.tensor.matmul(
                        gin[:], lhsT=cI32p[:], rhs=cZ2G[:, 2 * H:],
                        start=True, stop=False,
                    )
                    # ---- gather pe rows = lrelu(emb)[prev]
                    pe = sb.tile([B, H], F32, tag="pe")
                    nc.gpsimd.indirect_dma_start(
                        out=pe[:], out_offset=None, in_=d_e2[:],
                        in_offset=IndirectOffsetOnAxis(ap=prev_ap, axis=0),
                    )
                    trP = ps_tr.tile([128, 4, B], F32, tag="trP")
                    for k in range(4):
                        nc.tensor.transpose(
                            trP[:, k, :], pe[:, ts(k, 128)], cIdent[:B, :B]
                        )
                    peR = sb.tile([128, 4, B], F32, tag="peR")
                    nc.vector.tensor_copy(peR[:], trP[:])
                    # BN2 stats (over batch=free dim) per k-tile
                    st2 = sb.tile([128, 4, 6], F32, tag="st2")
                    mv2 = sb.tile([128, 4, 2], F32, tag="mv2")
                    for k in range(4):
                        nc.vector.bn_stats(st2[:, k, :], peR[:, k, :])
                        nc.vector.bn_aggr(mv2[:, k, :], st2[:, k, :])
                    sd2 = sb.tile([128, 4], F32, tag="sd2")
                    nc.scalar.activation(
                        sd2[:],
                        mv2[:, :, 1:2].rearrange("p a b -> p (a b)"),
                        AF.Sqrt, bias=cEps[:],
                    )
                    nc.vector.reciprocal(sd2[:], sd2[:])
                    nc.vector.tensor_mul(sd2[:], sd2[:], cBn2g[:])
                    peT = sb.tile([128, 4, B], F32, tag="peT")
                    for k in range(4):
                        nc.vector.tensor_scalar(
                            out=peT[:, k, :], in0=peR[:, k, :],
                            scalar1=mv2[:, k, 0:1], scalar2=sd2[:, k:k + 1],
                            op0=ALU.subtract, op1=ALU.mult,
                        )
                        if have_bn2b:
                            nc.vector.tensor_scalar_add(
                                peT[:, k, :], peT[:, k, :],
                                cBn2b[:, k:k + 1],
                            )
                    # ---- A: gi_pe accumulates into gh rz cols / gin
                    for k in range(4):
                        for n in range(2):
                            nc.tensor.matmul(
                                gh[0:B, ts(n, 512)], lhsT=peT[:, k, :],
                                rhs=cWl[:, k, ts(n, 512)],
                                start=False, stop=False,
                            )
                        nc.tensor.matmul(
                            gin[0:B, :], lhsT=peT[:, k, :],
                            rhs=cWl[:, k, 2 * H:],
                            start=False, stop=False,
                        )
                    # gim[t] injections close each accumulation group (they
                    # write the full 64 rows; mask zeroes the h-rows)
                    for n in range(2):
                        nc.tensor.matmul(
                            gh[:, ts(n, 512)], lhsT=cMask[:],
                            rhs=gmt[:, ts(n, 512)], start=False, stop=True,
                        )
                    nc.tensor.matmul(
                        gin[:], lhsT=cMask[:], rhs=gmt[:, 2 * H:],
                        start=False, stop=True,
                    )
                    # ---- GRU elementwise
                    rz = sb.tile([2 * B, 2 * H], F32, tag="rz")
                    nc.scalar.activation(rz[:], gh[:, 0:2 * H], AF.Sigmoid)
                    t1 = sb.tile([2 * B, H], F32, tag="t1")
                    if cBhhN is not None:
                        nc.vector.tensor_add(t1[:], gh[:, 2 * H:], cBhhN[:])
                        nc.vector.tensor_mul(t1[:], rz[:, 0:H], t1[:])
                    else:
                        nc.vector.tensor_tensor(
                            out=t1[:], in0=rz[:, 0:H], in1=gh[:, 2 * H:],
                            op=ALU.mult,
                        )
                    npre = sb.tile([2 * B, H], F32, tag="npre")
                    nc.vector.tensor_tensor(
                        out=npre[:], in0=t1[:], in1=gin[:], op=ALU.add
                    )
                    nt = sb.tile([2 * B, H], F32, tag="nt")
                    nc.scalar.activation(nt[:], npre[:], AF.Tanh)
                    dd = sb.tile([2 * B, H], F32, tag="dd")
                    nc.vector.tensor_tensor(
                        out=dd[:], in0=cSt[:], in1=nt[:], op=ALU.subtract
                    )
                    t2 = sb.tile([2 * B, H], F32, tag="t2")
                    nc.vector.tensor_tensor(
                        out=t2[:], in0=dd[:], in1=rz[:, H:2 * H], op=ALU.mult
                    )
                    nc.vector.tensor_tensor(
                        out=cSt[:], in0=t2[:], in1=nt[:], op=ALU.add
                    )
                    # ---- transpose state; lrelu; BN3
                    trS = ps_tr.tile([128, 4, 2 * B], F32, tag="trS")
                    for k in range(4):
                        nc.tensor.transpose(
                            trS[:, k, :], cSt[:, ts(k, 128)],
                            cIdent[:2 * B, :2 * B]
                        )
                    for k in range(4):
                        nc.vector.tensor_copy(cStT[:, k, :], trS[:, k, :])
                    lr = sb.tile([128, 4, 2 * B], F32, tag="lr")
                    lra = sb.tile([128, 4, 2 * B], F32, tag="lra")
                    nc.vector.tensor_scalar_mul(lra[:], trS[:], SLOPE)
                    nc.vector.tensor_tensor(
                        out=lr[:], in0=trS[:], in1=lra[:], op=ALU.max
                    )
                    st3 = sb.tile([128, 8, 6], F32, tag="st3")
                    mv3 = sb.tile([128, 8, 2], F32, tag="mv3")
                    for k in range(4):
                        for hf in range(2):
                            kh = 2 * k + hf
                            nc.vector.bn_stats(
                                st3[:, kh, :], lr[:, k, ts(hf, B)]
                            )
                            nc.vector.bn_aggr(mv3[:, kh, :], st3[:, kh, :])
                    sd3 = sb.tile([128, 8], F32, tag="sd3")
                    nc.scalar.activation(
                        sd3[:],
                        mv3[:, :, 1:2].rearrange("p a b -> p (a b)"),
                        AF.Sqrt, bias=cEps[:],
                    )
                    nc.vector.reciprocal(sd3[:], sd3[:])
                    nc.vector.tensor_mul(sd3[:], sd3[:], cBn3g8[:])
                    xT = sb.tile([128, 4, 2 * B], F32, tag="xT")
                    for k in range(4):
                        for hf in range(2):
                            kh = 2 * k + hf
                            nc.vector.tensor_scalar(
                                out=xT[:, k, ts(hf, B)],
                                in0=lr[:, k, ts(hf, B)],
                                scalar1=mv3[:, kh, 0:1],
                                scalar2=sd3[:, kh:kh + 1],
                                op0=ALU.subtract, op1=ALU.mult,
                            )
                            if have_bn3b:
                                nc.vector.tensor_scalar_add(
                                    xT[:, k, ts(hf, B)], xT[:, k, ts(hf, B)],
                                    cBn3b8[:, kh:kh + 1],
                                )
                    # ---- C: logits
                    lg = ps_log.tile([2 * B, O], F32, tag="lg")
                    for k in range(4):
                        for n in range(2):
                            nc.tensor.matmul(
                                lg[:, ts(n, 512)], lhsT=xT[:, k, :],
                                rhs=cHo[:, k, ts(n, 512)],
                                start=(k == 0), stop=(k == 3),
                            )
                    logits = sb.tile([2 * B, O], F32, tag="logits")
                    if cH2ob is not None:
                        nc.vector.tensor_add(logits[:], lg[:], cH2ob[:])
                    else:
                        nc.vector.tensor_copy(logits[:], lg[:])
                    mx = sb.tile([2 * B, 8], F32, tag="mx")
                    ix = sb.tile([2 * B, 8], U32, tag="ix")
                    nc.vector.max_with_indices(mx[:], ix[:], logits[:])
                    prev_ap = ix[0:B, 0:1]
                    nm = sb.tile([2 * B, 1], F32, tag="nm")
                    nc.vector.tensor_scalar_mul(nm[:], mx[:, 0:1], -1.0)
                    pr = sbl.tile([2 * B, O], BF16, tag="pr")
                    nc.scalar.activation(pr[:], logits[:], AF.Exp, bias=nm[:])
                    ph_stores[t] = nc.sync.dma_start(
                        d_ph[ts(t, B), :], pr[0:B, :]
                    ).ins
                    pm_stores[t] = nc.sync.dma_start(
                        d_pm[ts(t, B), :], pr[B:2 * B, :]
                    ).ins

            # =================== PHASE 2 ===================
            with (
                tc.tile_pool(name="p2", bufs=2) as p2,
                tc.tile_pool(name="s2", bufs=3) as s2,
                tc.tile_pool(name="ps2", bufs=2, space="PSUM") as ps2,
            ):
                CHUNK = 512
                r0 = 0
                while r0 < R:
                    ck = min(CHUNK, R - r0)
                    t_last = (r0 + ck) // B - 1  # last scan step this needs
                    phT = p2.tile([128, 8, CHUNK], BF16, tag="phT")
                    pmT = p2.tile([128, 8, CHUNK], BF16, tag="pmT")
                    for k in range(8):
                        i1 = nc.sync.dma_start_transpose(
                            phT[:, k, :ck], d_ph[ds(r0, ck), ts(k, 128)]
                        )
                        add_dep_helper(i1.ins, ph_stores[t_last],
                                       reason="phase2 reads scan probs")
                        i2 = nc.sync.dma_start_transpose(
                            pmT[:, k, :ck], d_pm[ds(r0, ck), ts(k, 128)]
                        )
                        add_dep_helper(i2.ins, pm_stores[t_last],
                                       reason="phase2 reads scan probs")
                    for mt0 in range(0, ck, 128):
                        mp = min(128, ck - mt0)
                        pu = ps2.tile([128, O], F32, tag="pu")
                        pv = ps2.tile([128, O], F32, tag="pv")
                        for k in range(8):
                            for n in range(2):
                                nc.tensor.matmul(
                                    pu[:mp, ts(n, 512)],
                                    lhsT=phT[:, k, ds(mt0, mp)],
                                    rhs=cLw[:, k, ts(n, 512)],
                                    start=(k == 0), stop=(k == 7),
                                )
                        for k in range(8):
                            for n in range(2):
                                nc.tensor.matmul(
                                    pv[:mp, ts(n, 512)],
                                    lhsT=pmT[:, k, ds(mt0, mp)],
                                    rhs=cLw[:, k, ts(n, 512)],
                                    start=(k == 0), stop=(k == 7),
                                )
                        u = s2.tile([128, O], F32, tag="u")
                        v = s2.tile([128, O], F32, tag="v")
                        ua = s2.tile([128, O], F32, tag="ua")
                        if cLinb is not None:
                            nc.vector.tensor_add(u[:mp], pu[:mp], cLinb[:mp])
                            nc.vector.tensor_scalar_mul(
                                ua[:mp], u[:mp], SLOPE)
                            nc.vector.tensor_tensor(
                                out=u[:mp], in0=u[:mp], in1=ua[:mp],
                                op=ALU.max)
                            nc.vector.tensor_add(v[:mp], pv[:mp], cLinb[:mp])
                            nc.vector.tensor_scalar_mul(
                                ua[:mp], v[:mp], SLOPE)
                            nc.vector.tensor_tensor(
                                out=v[:mp], in0=v[:mp], in1=ua[:mp],
                                op=ALU.max)
                        else:
                            nc.vector.tensor_scalar_mul(
                                ua[:mp], pu[:mp], SLOPE)
                            nc.vector.tensor_tensor(
                                out=u[:mp], in0=pu[:mp], in1=ua[:mp],
                                op=ALU.max)
                            nc.vector.tensor_scalar_mul(
                                ua[:mp], pv[:mp], SLOPE)
                            nc.vector.tensor_tensor(
                                out=v[:mp], in0=pv[:mp], in1=ua[:mp],
                                op=ALU.max)
                        uv = s2.tile([128, O], F32, tag="uv")
                        nc.vector.tensor_tensor(
                            out=uv[:mp], in0=u[:mp], in1=v[:mp], op=ALU.mult
                        )
                        u2 = s2.tile([128, O], F32, tag="u2")
                        nc.scalar.activation(u2[:mp], u[:mp], AF.Square)
                        v2 = s2.tile([128, O], F32, tag="v2")
                        nc.scalar.activation(v2[:mp], v[:mp], AF.Square)
                        dot = s2.tile([128, OSH], F32, tag="dot")
                        nc.vector.tensor_reduce(
                            out=dot[:mp],
                            in_=uv[:mp].rearrange("p (o h) -> p o h", h=HEADS),
                            axis=mybir.AxisListType.X, op=ALU.add,
                        )
                        nu = s2.tile([128, OSH], F32, tag="nu")
                        nc.vector.tensor_reduce(
                            out=nu[:mp],
                            in_=u2[:mp].rearrange("p (o h) -> p o h", h=HEADS),
                            axis=mybir.AxisListType.X, op=ALU.add,
                        )
                        nv = s2.tile([128, OSH], F32, tag="nv")
                        nc.vector.tensor_reduce(
                            out=nv[:mp],
                            in_=v2[:mp].rearrange("p (o h) -> p o h", h=HEADS),
                            axis=mybir.AxisListType.X, op=ALU.add,
                        )
                        den = s2.tile([128, OSH], F32, tag="den")
                        nc.vector.tensor_tensor(
                            out=den[:mp], in0=nu[:mp], in1=nv[:mp],
                            op=ALU.mult,
                        )
                        nc.scalar.activation(den[:mp], den[:mp], AF.Sqrt)
                        nc.vector.tensor_scalar_max(
                            den[:mp], den[:mp], EPS_COS
                        )
                        nc.vector.reciprocal(den[:mp], den[:mp])
                        cs = s2.tile([128, OSH], F32, tag="cs")
                        nc.vector.tensor_tensor(
                            out=cs[:mp], in0=dot[:mp], in1=den[:mp],
                            op=ALU.mult,
                        )
                        nc.sync.dma_start(d_out[ds(r0 + mt0, mp), :], cs[:mp])
                    r0 += ck

    return nc


_CACHE = {}


def _build_inputs(z, z2h_w, z2h_b, bn1_g, bn1_b, emb, bn2_g, bn2_b,
                  w_ih, w_hh, b_ih, b_hh, h2o_w, h2o_b, bn3_g, bn3_b,
                  lin_w, lin_b, memory, perm, T):
    f32 = np.float32
    z = np.asarray(z, f32)

    a = _lrelu_np(z @ np.asarray(z2h_w, f32).T + np.asarray(z2h_b, f32))
    mu, var = a.mean(0), a.var(0)
    z1 = (a - mu) / np.sqrt(var + EPS_BN) * np.asarray(bn1_g, f32) \
        + np.asarray(bn1_b, f32)                       # (B, H)

    E2 = _lrelu_np(np.asarray(emb, f32))               # (O, H)
    mem_rows = np.asarray(memory, f32)[0][np.asarray(perm).astype(np.int64)]
    xs = mem_rows[:T]                                  # (T, S)
    w_ih = np.asarray(w_ih, f32)
    b_ih = np.asarray(b_ih, f32)
    b_hh = np.asarray(b_hh, f32)
    gim = xs @ w_ih.T + b_ih                           # (T, G)

    bn2_g = np.asarray(bn2_g, f32)
    bn2_b = np.asarray(bn2_b, f32)
    mu2, var2 = z1.mean(0), z1.var(0)
    BNz = (z1 - mu2) / np.sqrt(var2 + EPS_BN) * bn2_g[H:] + bn2_b[H:]
    Z2G = BNz @ w_ih[:, H:].T + b_ih                   # (B, G)

    # fold b_hh's r/z parts into the input-side constants (rz = gi + gh)
    Z2G = Z2G.copy()
    Z2G[:, :2 * H] += b_hh[:2 * H]
    gim[:, :2 * H] += b_hh[:2 * H]

    flags = (
        bool(np.any(b_hh[2 * H:] != 0)),
        bool(np.any(np.asarray(h2o_b, f32) != 0)),
        bool(np.any(np.asarray(lin_b, f32) != 0)),
        bool(np.any(bn2_b[:H] != 0)),
        bool(np.any(np.asarray(bn3_b, f32) != 0)),
    )

    def tile_kT(w):  # (K, N) -> (128, K//128, N), K on partitions
        K, N = w.shape
        return np.ascontiguousarray(
            w.reshape(K // 128, 128, N).transpose(1, 0, 2))

    def perk(v):  # (512,) -> (128, 4)
        return np.ascontiguousarray(v.reshape(4, 128).T)

    bn3_g = np.asarray(bn3_g, f32)
    bn3_b = np.asarray(bn3_b, f32)
    st0 = np.concatenate([z1, z1], 0)                  # (64, H)
    lin_w = np.asarray(lin_w, f32)

    common = {
        "wl": tile_kT(w_ih[:, :H].T),
        "whh": tile_kT(np.asarray(w_hh, f32).T),
        "ho": tile_kT(np.asarray(h2o_w, f32).T),
        "z2g": np.ascontiguousarray(Z2G.astype(f32)),
        "gim": np.ascontiguousarray(gim.astype(f32)),
        "e2": np.ascontiguousarray(E2),
        "st0": np.ascontiguousarray(st0),
        "st0t": tile_kT(st0.T),
        "bn2g": perk(bn2_g[:H]), "bn2b": perk(bn2_b[:H]),
        "bn3g8": np.repeat(perk(bn3_g), 2, axis=1),
        "bn3b8": np.repeat(perk(bn3_b), 2, axis=1),
        "maskm": np.concatenate(
            [np.zeros((1, B), f32), np.ones((1, B), f32)], 1),
        "i32p": np.concatenate(
            [np.eye(B, dtype=f32), np.zeros((B, B), f32)], 1),
    }
    if flags[0]:
        common["bhhn"] = b_hh[2 * H:].reshape(1, H).astype(f32)
    if flags[1]:
        common["h2ob"] = np.asarray(h2o_b, f32).reshape(1, O)
    in_maps = []
    for c in range(NCORES):
        m = dict(common)
        lw_c = lin_w[c * O:(c + 1) * O, :]             # (1024, 1024)
        m["linw"] = tile_kT(lw_c.T).astype(ml_dtypes.bfloat16)
        if flags[2]:
            m["linb"] = np.asarray(lin_b, f32)[c * O:(c + 1) * O].reshape(1, O)
        in_maps.append(m)
    return in_maps, flags


def kernel(z, z2h_w, z2h_b, bn1_g, bn1_b, emb, bn2_g, bn2_b,
           w_ih, w_hh, b_ih, b_hh, h2o_w, h2o_b, bn3_g, bn3_b,
           lin_w, lin_b, memory, perm, num_steps, temperature):
    T = int(num_steps)
    in_maps, flags = _build_inputs(
        z, z2h_w, z2h_b, bn1_g, bn1_b, emb, bn2_g, bn2_b,
        w_ih, w_hh, b_ih, b_hh, h2o_w, h2o_b, bn3_g, bn3_b,
        lin_w, lin_b, memory, perm, T)

    key = (T, flags)
    if key not in _CACHE:
        nc = build_program(T, flags)
        _split_multiwait_ctrl(nc)
        _CACHE[key] = nc
    nc = _CACHE[key]

    res = run_bass_kernel_spmd(nc, in_maps, list(range(NCORES)))

    out = np.empty((B, T, O), np.float32)
    for c in range(NCORES):
        oc = res.results[c]["out"]                     # (R, 128)
        out[:, :, c * OSH:(c + 1) * OSH] = \
            oc.reshape(T, B, OSH).swapaxes(0, 1)
    return out
